# revision 7
# baseline (speedup 1.0000x reference)
"""Trainium2 kernel for nn_CPNet_33165737460025 (retrieval_knn).

Pure data parallel: one sample per NeuronCore (B=8 over 8 cores).

Device (per core, per sample):
  - d' = (||p1-p2||^2 + EPS)/FACT via one fp32 k=5 PE matmul (bilinear expansion)
  - s = 1/d' = FACT/(d+EPS) via reciprocal_approx_fast (18-bit, enough: softmax
    is invariant to per-row logit shifts up to exp noise)
  - bias = -rowmax(s) via tensor_reduce(max, negate)
  - E = exp(s + bias) -> fp16 on the scalar engine (per-partition bias, free)
  - E^T per 128-chunk via DMA xbar transpose (fp16)
  - pcn_raw[c, n1] = sum_n2 E * pc2[c] via fp16 PE matmuls (pc2 row3 = ones
    gives the softmax row-sum as channel 3)
Host: tiny O(N) tail — outlier rejection, DLT normal matrix, 8192x12 SVD,
  4x4 inverse, rotation normalization, quaternion.
"""
import numpy as np

B = 8
N = 4096
NT = N // 128  # 32 n1-tiles
EPS = 1e-05
EPS_T = 2e-06  # softmax-logit shift: s = FACT/(d + EPS_T) instead of FACT/clip(d, EPS).
               # Bias-free vs clip for all but sub-EPS pairs (one-hot either way);
               # keeps d' > 0 under fp32 cancellation so 1/d' never blows up.
FACT = 2.0

_cache = {}


def _build_device_program():
    import concourse.bass as bass
    import concourse.tile as tile
    from concourse import bacc, mybir

    F32 = mybir.dt.float32
    F16 = mybir.dt.float16

    nc = bacc.Bacc("TRN2", target_bir_lowering=False, debug=False, num_devices=B)
    l1 = nc.dram_tensor("l1", [5, N], F32, kind="ExternalInput").ap()
    q2 = nc.dram_tensor("q2", [5, N], F32, kind="ExternalInput").ap()
    pc2t = nc.dram_tensor("pc2t", [128, NT * 8], F16, kind="ExternalInput").ap()
    pcn = nc.dram_tensor("pcn", [8, N], F32, kind="ExternalOutput").ap()

    with tile.TileContext(nc) as tc:
        with (
            tc.tile_pool(name="const", bufs=1) as constp,
            tc.tile_pool(name="s", bufs=2) as sp,
            tc.tile_pool(name="e", bufs=2) as ep,
            tc.tile_pool(name="et", bufs=2) as etp,
            tc.tile_pool(name="stat", bufs=3) as statp,
            tc.tile_pool(name="psd", bufs=3, space="PSUM") as psd,
            tc.tile_pool(name="pspv", bufs=2, space="PSUM") as pspv,
        ):
            l1_sb = constp.tile([5, N], F32)
            nc.sync.dma_start(l1_sb[:], l1)
            q2_sb = constp.tile([5, N], F32)
            nc.sync.dma_start(q2_sb[:], q2)
            p2t_sb = constp.tile([128, NT * 8], F16)
            nc.sync.dma_start(p2t_sb[:], pc2t)
            pcn_sb = constp.tile([8, N], F32)

            for t in range(NT):
                s_sb = sp.tile([128, N], F32)
                for g in range(4):
                    pd = psd.tile([128, 1024], F32)
                    for j in range(2):
                        nc.tensor.matmul(
                            pd[:, j * 512:(j + 1) * 512],
                            l1_sb[:, t * 128:(t + 1) * 128],
                            q2_sb[:, g * 1024 + j * 512: g * 1024 + (j + 1) * 512],
                            start=True, stop=True,
                        )
                    nc.vector.reciprocal_approx_fast(
                        s_sb[:, g * 1024:(g + 1) * 1024], pd[:]
                    )
                bias_t = statp.tile([128, 1], F32)
                nc.vector.tensor_reduce(
                    bias_t[:], s_sb[:], axis=mybir.AxisListType.X,
                    op=mybir.AluOpType.max, negate=True,
                )
                e_sb = ep.tile([128, N], F16)
                nc.scalar.activation(
                    e_sb[:], s_sb[:], mybir.ActivationFunctionType.Exp,
                    bias=bias_t[:, 0:1], scale=1.0,
                )
                et_sb = etp.tile([128, NT, 128], F16)
                nc.sync.dma_start_transpose(et_sb[:], e_sb[:])
                pv = pspv.tile([8, 128], F32)
                for k in range(NT):
                    nc.tensor.matmul(
                        pv[:], p2t_sb[:, k * 8:(k + 1) * 8], et_sb[:, k, :],
                        start=(k == 0), stop=(k == NT - 1),
                    )
                nc.scalar.copy(pcn_sb[:, t * 128:(t + 1) * 128], pv[:])

            nc.sync.dma_start(pcn, pcn_sb[:])

    nc.compile()
    return nc


def _get_nc():
    if "nc" not in _cache:
        _cache["nc"] = _build_device_program()
    return _cache["nc"]


def _prep_inputs(pc1, pc2):
    """Per-sample host prep of the three device tensors."""
    sq = np.float32(1.0 / np.sqrt(FACT))
    maps = []
    ones = np.ones(N, dtype=np.float32)
    for b in range(B):
        p1 = pc1[b]
        p2 = pc2[b]
        p1sq = (p1[0] * p1[0] + p1[1] * p1[1] + p1[2] * p1[2]).astype(np.float32)
        p2sq = (p2[0] * p2[0] + p2[1] * p2[1] + p2[2] * p2[2]).astype(np.float32)
        l1 = np.stack([p1[0], p1[1], p1[2], ones, p1sq]).astype(np.float32) * sq
        q2 = np.stack(
            [-2.0 * p2[0], -2.0 * p2[1], -2.0 * p2[2],
             p2sq + np.float32(EPS_T), ones]
        ).astype(np.float32) * sq
        # fp16 hi/lo split of pc2 so the fp16 PV matmul reaches ~fp32 accuracy:
        # pc2t[p, k*8+c] = X[c, k*128+p], X = [pc2_hi(4); pc2_lo(4)]
        xh = p2.astype(np.float16)
        xl = (p2.astype(np.float64) - xh.astype(np.float64)).astype(np.float16)
        X = np.concatenate([xh, xl], axis=0)  # (8, N)
        pc2t = np.ascontiguousarray(
            X.T.reshape(NT, 128, 8).transpose(1, 0, 2).reshape(128, NT * 8)
        )
        maps.append({"l1": l1, "q2": q2, "pc2t": pc2t})
    return maps


# ---------------- host tail (numpy, float64 internally) ----------------

def _norm_tf(pc):
    std = pc[:3].std(axis=1, ddof=1)
    mean = pc[:3].mean(axis=1)
    T = np.zeros((4, 4))
    T[0, 0], T[1, 1], T[2, 2] = 1.0 / std
    T[:3, 3] = -mean / std
    T[3, 3] = 1.0
    return T


def _normalize_rotmat(R):
    z = R[:, 2] / np.linalg.norm(R[:, 2])
    y = R[:, 1]
    x = np.cross(y, z)
    x = x / np.linalg.norm(x)
    y = np.cross(z, x)
    return np.stack([x, y, z], axis=1)


def _rot_to_quat(R):
    r00, r11, r22 = R[0, 0], R[1, 1], R[2, 2]
    qw = 0.5 * np.sqrt(max(1e-12, 1.0 + r00 + r11 + r22))
    qx = 0.5 * np.sqrt(max(0.0, 1.0 + r00 - r11 - r22))
    qy = 0.5 * np.sqrt(max(0.0, 1.0 - r00 + r11 - r22))
    qz = 0.5 * np.sqrt(max(0.0, 1.0 - r00 - r11 + r22))
    if R[2, 1] - R[1, 2] < 0:
        qx = -qx
    if R[0, 2] - R[2, 0] < 0:
        qy = -qy
    if R[1, 0] - R[0, 1] < 0:
        qz = -qz
    return np.array([qw, qx, qy, qz])


def _post_sample(pc1_b, raw8):
    """pc1_b (4,N) f32; raw8 (8,N) device output: rows 0-3 hi sums, 4-7 lo sums."""
    pc1_b = pc1_b.astype(np.float64)
    raw = raw8[0:4].astype(np.float64) + raw8[4:8].astype(np.float64)
    pcn = np.empty_like(pc1_b)
    pcn[:3] = raw[:3] / raw[3]
    pcn[3] = 1.0

    dist = np.linalg.norm(pc1_b - pcn, axis=0)
    mask = (dist < dist.mean() + EPS).astype(np.float64)

    T2 = _norm_tf(pcn)
    pc2n = T2 @ pcn
    T1 = _norm_tf(pc1_b)
    pc1n = T1 @ pc1_b

    a = pc1n.T  # (N, 4)
    z4 = np.zeros((N, 4))
    row0 = np.concatenate([z4, -pc2n[2][:, None] * a, pc2n[1][:, None] * a], axis=1)
    row1 = np.concatenate([pc2n[2][:, None] * a, z4, -pc2n[0][:, None] * a], axis=1)
    M = (np.stack([row0, row1], axis=1) * mask[:, None, None]).reshape(2 * N, 12)

    _, _, Vh = np.linalg.svd(M, full_matrices=False)
    p = Vh[-1]
    if p[10] < 0:
        p = -p
    p = p / np.sqrt(p[8] ** 2 + p[9] ** 2 + p[10] ** 2)

    T = np.zeros((4, 4))
    T[:3, :] = p.reshape(3, 4)
    T[3, 3] = 1.0
    T = np.linalg.inv(T2) @ T @ T1
    R = _normalize_rotmat(T[:3, :3])
    T[:3, :3] = R
    return T, _rot_to_quat(R), T[:3, 3].copy()


def kernel(pc1, pc2):
    from concourse.bass_utils import run_bass_kernel_spmd

    pc1 = np.asarray(pc1, dtype=np.float32)
    pc2 = np.asarray(pc2, dtype=np.float32)
    nc = _get_nc()
    in_maps = _prep_inputs(pc1, pc2)
    res = run_bass_kernel_spmd(nc, in_maps, list(range(B)))

    Ts = np.empty((B, 4, 4), dtype=np.float32)
    qs = np.empty((B, 4), dtype=np.float32)
    ts = np.empty((B, 3), dtype=np.float32)
    for b in range(B):
        T, q, t = _post_sample(pc1[b], res.results[b]["pcn"])
        Ts[b], qs[b], ts[b] = T, q, t
    return Ts, qs, ts


# revision 13
# speedup vs baseline: 135.8717x; 135.8717x over previous
"""Trainium2 kernel for nn_CPNet_33165737460025 (retrieval_knn).

Pure data parallel: one sample per NeuronCore (B=8 over 8 cores).

Device (per core, per sample):
  - d' = (||p1-p2||^2 + EPS)/FACT via one fp32 k=5 PE matmul (bilinear expansion)
  - s = 1/d' = FACT/(d+EPS) via reciprocal_approx_fast (18-bit, enough: softmax
    is invariant to per-row logit shifts up to exp noise)
  - bias = -rowmax(s) via tensor_reduce(max, negate)
  - E = exp(s + bias) -> fp16 on the scalar engine (per-partition bias, free)
  - E^T per 128-chunk via DMA xbar transpose (fp16)
  - pcn_raw[c, n1] = sum_n2 E * pc2[c] via fp16 PE matmuls (pc2 row3 = ones
    gives the softmax row-sum as channel 3)
Host: tiny O(N) tail — outlier rejection, DLT normal matrix, 8192x12 SVD,
  4x4 inverse, rotation normalization, quaternion.
"""
import numpy as np

B = 8
N = 4096
NT = N // 128  # 32 n1-tiles
EPS = 1e-05
EPS_T = 2e-06  # softmax-logit shift: s = FACT/(d + EPS_T) instead of FACT/clip(d, EPS).
               # Bias-free vs clip for all but sub-EPS pairs (one-hot either way);
               # keeps d' > 0 under fp32 cancellation so 1/d' never blows up.
FACT = 2.0

_cache = {}


def _build_device_program(repeat=1):
    import concourse.bass as bass
    import concourse.tile as tile
    from concourse import bacc, mybir

    F32 = mybir.dt.float32
    F16 = mybir.dt.float16

    nc = bacc.Bacc("TRN2", target_bir_lowering=False, debug=False, num_devices=B)
    l1 = nc.dram_tensor("l1", [5, N], F32, kind="ExternalInput").ap()
    q2 = nc.dram_tensor("q2", [5, N], F32, kind="ExternalInput").ap()
    pc2t = nc.dram_tensor("pc2t", [128, NT * 8], F16, kind="ExternalInput").ap()
    pcn = nc.dram_tensor("pcn", [8, N], F32, kind="ExternalOutput").ap()

    from concourse import masks

    with tile.TileContext(nc) as tc:
        with (
            tc.tile_pool(name="const", bufs=1) as constp,
            tc.tile_pool(name="s", bufs=2) as sp,
            tc.tile_pool(name="e", bufs=3) as ep,
            tc.tile_pool(name="et", bufs=3) as etp,
            tc.tile_pool(name="stat", bufs=3) as statp,
            tc.tile_pool(name="psd", bufs=2, space="PSUM") as psd,
            tc.tile_pool(name="pst", bufs=2, space="PSUM") as pst,
            tc.tile_pool(name="pspv", bufs=2, space="PSUM") as pspv,
        ):
            l1_sb = constp.tile([5, N], F32)
            nc.sync.dma_start(l1_sb[:], l1)
            q2_sb = constp.tile([5, N], F32)
            nc.sync.dma_start(q2_sb[:], q2)
            p2t_sb = constp.tile([128, NT * 8], F16)
            nc.sync.dma_start(p2t_sb[:], pc2t)
            pcn_sb = constp.tile([8, N], F32)
            ident = constp.tile([128, 128], F16)
            masks.make_identity(nc, ident[:])

            # Software pipeline: transposes for tile t-1, PV for tile t-2, so
            # the PE FIFO never stalls behind the recip->exp dependency chain.
            total = NT * repeat
            es, ets = {}, {}
            for step in range(total + 2):
                if step < total:
                    t = step % NT
                    s_sb = sp.tile([128, N], F32)
                    for g in range(4):
                        pd = psd.tile([128, 1024], F32)
                        for j in range(2):
                            nc.tensor.matmul(
                                pd[:, j * 512:(j + 1) * 512],
                                l1_sb[:, t * 128:(t + 1) * 128],
                                q2_sb[:, g * 1024 + j * 512: g * 1024 + (j + 1) * 512],
                                start=True, stop=True,
                            )
                        nc.vector.reciprocal_approx_fast(
                            s_sb[:, g * 1024:(g + 1) * 1024], pd[:]
                        )
                    bias_t = statp.tile([128, 1], F32)
                    nc.vector.tensor_reduce(
                        bias_t[:], s_sb[:], axis=mybir.AxisListType.X,
                        op=mybir.AluOpType.max, negate=True,
                    )
                    e_sb = ep.tile([128, N], F16)
                    nc.scalar.activation(
                        e_sb[:], s_sb[:], mybir.ActivationFunctionType.Exp,
                        bias=bias_t[:, 0:1], scale=1.0,
                    )
                    es[step] = e_sb
                if 0 <= step - 1 < total:
                    # PE-transpose E (fp16) into PSUM, copy out in 1024-wide
                    # batches alternating DVE/ACT.
                    e_sb = es.pop(step - 1)
                    et_sb = etp.tile([128, NT, 128], F16)
                    ets[step - 1] = et_sb
                    for g in range(4):
                        pt = pst.tile([128, 1024], F16)
                        for k in range(8):
                            c = g * 8 + k
                            nc.tensor.transpose(
                                pt[:, k * 128:(k + 1) * 128],
                                e_sb[:, c * 128:(c + 1) * 128], ident[:],
                            )
                        dst = et_sb[:, g * 8:(g + 1) * 8, :]
                        if g % 2 == 0:
                            nc.vector.tensor_copy(dst, pt[:])
                        else:
                            nc.scalar.copy(dst, pt[:])
                if step >= 2:
                    sp_ = step - 2
                    tp = sp_ % NT
                    et_sb = ets.pop(sp_)
                    pv = pspv.tile([8, 128], F32)
                    for k in range(NT):
                        nc.tensor.matmul(
                            pv[:], p2t_sb[:, k * 8:(k + 1) * 8], et_sb[:, k, :],
                            start=(k == 0), stop=(k == NT - 1),
                        )
                    nc.scalar.copy(pcn_sb[:, tp * 128:(tp + 1) * 128], pv[:])

            nc.sync.dma_start(pcn, pcn_sb[:])

    nc.compile()
    return nc


def _get_nc(repeat=1):
    key = ("nc", repeat)
    if key not in _cache:
        _cache[key] = _build_device_program(repeat)
    return _cache[key]


def _prep_inputs(pc1, pc2):
    """Per-sample host prep of the three device tensors."""
    sq = np.float32(1.0 / np.sqrt(FACT))
    maps = []
    ones = np.ones(N, dtype=np.float32)
    for b in range(B):
        p1 = pc1[b]
        p2 = pc2[b]
        p1sq = (p1[0] * p1[0] + p1[1] * p1[1] + p1[2] * p1[2]).astype(np.float32)
        p2sq = (p2[0] * p2[0] + p2[1] * p2[1] + p2[2] * p2[2]).astype(np.float32)
        l1 = np.stack([p1[0], p1[1], p1[2], ones, p1sq]).astype(np.float32) * sq
        q2 = np.stack(
            [-2.0 * p2[0], -2.0 * p2[1], -2.0 * p2[2],
             p2sq + np.float32(EPS_T), ones]
        ).astype(np.float32) * sq
        # fp16 hi/lo split of pc2 so the fp16 PV matmul reaches ~fp32 accuracy:
        # pc2t[p, k*8+c] = X[c, k*128+p], X = [pc2_hi(4); pc2_lo(4)]
        xh = p2.astype(np.float16)
        xl = (p2.astype(np.float64) - xh.astype(np.float64)).astype(np.float16)
        X = np.concatenate([xh, xl], axis=0)  # (8, N)
        pc2t = np.ascontiguousarray(
            X.T.reshape(NT, 128, 8).transpose(1, 0, 2).reshape(128, NT * 8)
        )
        maps.append({"l1": l1, "q2": q2, "pc2t": pc2t})
    return maps


# ---------------- host tail (numpy, float64 internally) ----------------

def _norm_tf(pc):
    std = pc[:3].std(axis=1, ddof=1)
    mean = pc[:3].mean(axis=1)
    T = np.zeros((4, 4))
    T[0, 0], T[1, 1], T[2, 2] = 1.0 / std
    T[:3, 3] = -mean / std
    T[3, 3] = 1.0
    return T


def _normalize_rotmat(R):
    z = R[:, 2] / np.linalg.norm(R[:, 2])
    y = R[:, 1]
    x = np.cross(y, z)
    x = x / np.linalg.norm(x)
    y = np.cross(z, x)
    return np.stack([x, y, z], axis=1)


def _rot_to_quat(R):
    r00, r11, r22 = R[0, 0], R[1, 1], R[2, 2]
    qw = 0.5 * np.sqrt(max(1e-12, 1.0 + r00 + r11 + r22))
    qx = 0.5 * np.sqrt(max(0.0, 1.0 + r00 - r11 - r22))
    qy = 0.5 * np.sqrt(max(0.0, 1.0 - r00 + r11 - r22))
    qz = 0.5 * np.sqrt(max(0.0, 1.0 - r00 - r11 + r22))
    if R[2, 1] - R[1, 2] < 0:
        qx = -qx
    if R[0, 2] - R[2, 0] < 0:
        qy = -qy
    if R[1, 0] - R[0, 1] < 0:
        qz = -qz
    return np.array([qw, qx, qy, qz])


def _post_sample(pc1_b, raw8):
    """pc1_b (4,N) f32; raw8 (8,N) device output: rows 0-3 hi sums, 4-7 lo sums."""
    pc1_b = pc1_b.astype(np.float64)
    raw = raw8[0:4].astype(np.float64) + raw8[4:8].astype(np.float64)
    pcn = np.empty_like(pc1_b)
    pcn[:3] = raw[:3] / raw[3]
    pcn[3] = 1.0

    dist = np.linalg.norm(pc1_b - pcn, axis=0)
    mask = (dist < dist.mean() + EPS).astype(np.float64)

    T2 = _norm_tf(pcn)
    pc2n = T2 @ pcn
    T1 = _norm_tf(pc1_b)
    pc1n = T1 @ pc1_b

    a = pc1n.T  # (N, 4)
    z4 = np.zeros((N, 4))
    row0 = np.concatenate([z4, -pc2n[2][:, None] * a, pc2n[1][:, None] * a], axis=1)
    row1 = np.concatenate([pc2n[2][:, None] * a, z4, -pc2n[0][:, None] * a], axis=1)
    M = (np.stack([row0, row1], axis=1) * mask[:, None, None]).reshape(2 * N, 12)

    _, _, Vh = np.linalg.svd(M, full_matrices=False)
    p = Vh[-1]
    if p[10] < 0:
        p = -p
    p = p / np.sqrt(p[8] ** 2 + p[9] ** 2 + p[10] ** 2)

    T = np.zeros((4, 4))
    T[:3, :] = p.reshape(3, 4)
    T[3, 3] = 1.0
    T = np.linalg.inv(T2) @ T @ T1
    R = _normalize_rotmat(T[:3, :3])
    T[:3, :3] = R
    return T, _rot_to_quat(R), T[:3, 3].copy()


def kernel(pc1, pc2):
    from concourse.bass_utils import run_bass_kernel_spmd

    pc1 = np.asarray(pc1, dtype=np.float32)
    pc2 = np.asarray(pc2, dtype=np.float32)
    nc = _get_nc()
    in_maps = _prep_inputs(pc1, pc2)
    res = run_bass_kernel_spmd(nc, in_maps, list(range(B)))

    Ts = np.empty((B, 4, 4), dtype=np.float32)
    qs = np.empty((B, 4), dtype=np.float32)
    ts = np.empty((B, 3), dtype=np.float32)
    for b in range(B):
        T, q, t = _post_sample(pc1[b], res.results[b]["pcn"])
        Ts[b], qs[b], ts[b] = T, q, t
    return Ts, qs, ts
